# revision 33
# baseline (speedup 1.0000x reference)
"""Chamfer-distance (CDLoss) Trainium2 Bass kernel.

Problem: srcs, tgts [B=8, D=3, N=4096] fp32.
  P[b,i,j] = |s_i|^2 + |t_j|^2 - 2 s_i.t_j
  out = min(P, axis=1).mean() + min(P, axis=2).mean()   (scalar fp32)

Strategy (data-parallel over B across 8 NeuronCores, one batch per core):
  Two "directions" per core, each a 4096x4096 implicit distance matrix:
    dir1: for each source i, min over targets j of d^2(s_i, t_j)
    dir2: for each target j, min over sources i of d^2(t_j, s_i)

  Matrix tiles are produced by TensorE matmuls with bf16 hi/lo-split
  features (K=18 rows; query and candidate norms are folded in, so PSUM
  holds d^2 >= 0 exactly to ~1e-6).  The 4 matmuls of each PSUM chunk go
  to 4 different PE row groups (tile_position) so LDWEIGHTS/MATMUL
  overlap and matmuls run concurrently.

  Row-min reduction is split between two engines:
    - "assisted" M-tiles: ScalarE casts PSUM fp32 -> fp16 SBUF (ACTIVATE
      Copy), then VectorE runs a tensor_tensor min tree in fp16 (2x packed
      mode, 2 outputs/cycle) + one small 1x reduce.
    - "pure" M-tiles: VectorE reduces PSUM fp32 directly at 1x.
  The ratio keeps both engines saturated.

  Per-core outputs are 2x[128, 32] row-min matrices; the host averages
  and combines across cores (query norms are already included).
"""

import numpy as np
import ml_dtypes

_BF16 = ml_dtypes.bfloat16

# Problem geometry (hardcoded per contest contract).
_B = 8
_D = 3
_N = 4096
_P = 128              # partitions / queries per M-tile
_K = 18               # feature rows (see _features)
_NCORES = 8
_CHUNK = 2048         # PSUM chunk columns (4 banks)

_prog_cache = {}

# test-harness knobs (the grading harness just calls kernel() and never
# touches these; default is the fast no-trace path)
TRACE = False
TRACE_CORES = [0]
LAST_RESULTS = None

# M-tile flavors: "full" = ACT casts both chunks, DVE runs the fp16 tree;
# "mixed" = ACT casts chunk0 while DVE reduces chunk1 from PSUM directly.
# Ratio tuned so both engines stay saturated (see _is_mixed).
_MIX_NUM, _MIX_DEN = 12, 32


def _is_mixed(m):
    return (m * _MIX_NUM) % _MIX_DEN < _MIX_NUM


def _build_program(n_pts=_N):
    import concourse.mybir as mybir
    import concourse.tile as tile
    from concourse import bacc

    P = _P
    MT = n_pts // P
    K = _K
    NCH = n_pts // _CHUNK          # psum chunks per M-tile
    f32 = mybir.dt.float32
    f16 = mybir.dt.float16
    bf16 = mybir.dt.bfloat16
    MIN = mybir.AluOpType.min

    nc = bacc.Bacc("TRN2", target_bir_lowering=False, debug=False,
                   num_devices=_NCORES)

    dram = {}
    for d in (1, 2):
        dram[f"w{d}"] = nc.dram_tensor(f"w{d}", [K, n_pts], bf16,
                                       kind="ExternalInput")
        dram[f"r{d}"] = nc.dram_tensor(f"r{d}", [K, n_pts], bf16,
                                       kind="ExternalInput")
        dram[f"out{d}"] = nc.dram_tensor(f"out{d}", [P, MT], f32,
                                         kind="ExternalOutput")

    with tile.TileContext(nc) as tc:
        with (
            tc.tile_pool(name="const", bufs=2) as cpool,
            tc.tile_pool(name="work", bufs=6) as wpool,
            tc.tile_pool(name="tree", bufs=6) as tpool,
            tc.tile_pool(name="acc", bufs=2) as apool,
            tc.tile_pool(name="psum", bufs=2, space="PSUM") as ppool,
        ):
            def halvings_and_reduce(cur, w, out_ap):
                """DVE: fp16 halving TTs (2x mode) then a small 1x
                reduce, [P, w] fp16 -> out_ap [P, 1] f32 (column min)."""
                while w > 512:
                    o = tpool.tile([P, w // 2], f16, tag=f"h{w}",
                                   name=f"h{w}")
                    nc.vector.tensor_tensor(
                        o[:], cur[:, :w // 2], cur[:, w // 2:], op=MIN)
                    cur, w = o, w // 2
                nc.vector.tensor_reduce(
                    out_ap, cur[:], axis=mybir.AxisListType.X, op=MIN)

            for d in (1, 2):
                # Inputs are compact [K, N]; replicate into the 4 PE row
                # groups with 4 small DMAs each.
                sbW = cpool.tile([128, n_pts], bf16, tag="sbW")
                sbR = cpool.tile([128, n_pts], bf16, tag="sbR")
                for g in range(4):
                    # direction 1: two HWDGE queues so the startup loads
                    # overlap; direction 2 loads go on the idle sync
                    # queue so they never interrupt ScalarE's casts.
                    r_eng = nc.scalar if d == 1 else nc.sync
                    nc.sync.dma_start(sbW[32 * g:32 * g + K, :],
                                      dram[f"w{d}"][:])
                    r_eng.dma_start(sbR[32 * g:32 * g + K, :],
                                    dram[f"r{d}"][:])
                sbOut = cpool.tile([P, MT], f32, tag="sbOut")

                for m in range(MT):
                    mixed = _is_mixed(m) and NCH == 2
                    starter = (d == 1 and m == 0 and NCH == 2)
                    if starter:
                        # First tile: 1024-col chunks (2 matmuls each) so
                        # the first ScalarE cast fires as early as
                        # possible after the input DMAs land.
                        sb = []
                        for h in range(4):
                            ps = ppool.tile([P, _CHUNK], f32, tag="ps")
                            for q in range(2):
                                g = (2 * h + q) % 4
                                col = 1024 * h + 512 * q
                                nc.tensor.matmul(
                                    ps[:, 512 * q:512 * (q + 1)],
                                    sbW[32 * g:32 * g + K, :P],
                                    sbR[32 * g:32 * g + K, col:col + 512],
                                    start=True, stop=True,
                                    tile_position=(32 * g, 0),
                                )
                            cast = wpool.tile([P, 1024], f16,
                                              tag="scast", name="scast")
                            nc.scalar.copy(cast[:], ps[:, 0:1024])
                            sb.append(cast)
                        lvl = 0
                        while len(sb) > 1:
                            nxt = []
                            for i in range(0, len(sb), 2):
                                o = tpool.tile([P, 1024], f16,
                                               tag=f"st{lvl}_{i}",
                                               name=f"st{lvl}_{i}")
                                nc.vector.tensor_tensor(
                                    o[:], sb[i][:], sb[i + 1][:], op=MIN)
                                nxt.append(o)
                            sb = nxt
                            lvl += 1
                        halvings_and_reduce(sb[0], 1024,
                                            sbOut[:, m:m + 1])
                        continue
                    chunks = []
                    for h in range(NCH):
                        ps = ppool.tile([P, _CHUNK], f32, tag="ps")
                        for q in range(4):
                            # row group rotates per 512-col matmul so
                            # LDWEIGHTS overlaps in-flight MATMULs and
                            # matmuls run concurrently on the PE array.
                            g = q
                            col = _CHUNK * h + 512 * q
                            nc.tensor.matmul(
                                ps[:, 512 * q:512 * (q + 1)],
                                sbW[32 * g:32 * g + K, m * P:(m + 1) * P],
                                sbR[32 * g:32 * g + K, col:col + 512],
                                start=True, stop=True,
                                tile_position=(32 * g, 0),
                            )
                        chunks.append(ps)

                    # Casts and PSUM reduces are split [1536|512] so the
                    # first op releases 3 of 4 PSUM banks early -- the
                    # next chunk's matmuls refill them while the small
                    # second op finishes (bank-granular deps).
                    SPL = 1536

                    if mixed:
                        # ACT casts chunk0; DVE reduces chunk1 from PSUM
                        # concurrently; combine the partial mins.
                        cast = wpool.tile([P, _CHUNK], f16, tag="cast0")
                        nc.scalar.copy(cast[:, :SPL], chunks[0][:, :SPL])
                        nc.scalar.copy(cast[:, SPL:], chunks[0][:, SPL:])
                        tmp = apool.tile([P, 3], f32, tag="tmp")
                        nc.vector.tensor_reduce(
                            tmp[:, 1:2], chunks[1][:, :SPL],
                            axis=mybir.AxisListType.X, op=MIN)
                        nc.vector.tensor_reduce(
                            tmp[:, 2:3], chunks[1][:, SPL:],
                            axis=mybir.AxisListType.X, op=MIN)
                        halvings_and_reduce(cast, _CHUNK, tmp[:, 0:1])
                        nc.vector.tensor_reduce(
                            sbOut[:, m:m + 1], tmp[:],
                            axis=mybir.AxisListType.X, op=MIN)
                    else:
                        sb = []
                        for h, ps in enumerate(chunks):
                            cast = wpool.tile([P, _CHUNK], f16,
                                              tag=f"cast{h}")
                            nc.scalar.copy(cast[:, :SPL], ps[:, :SPL])
                            nc.scalar.copy(cast[:, SPL:], ps[:, SPL:])
                            sb.append(cast)
                        # fold tile pairs at 2x, then halve
                        lvl = 0
                        while len(sb) > 1:
                            nxt = []
                            for i in range(0, len(sb), 2):
                                o = tpool.tile([P, _CHUNK], f16,
                                               tag=f"t{lvl}_{i}")
                                nc.vector.tensor_tensor(
                                    o[:], sb[i][:], sb[i + 1][:], op=MIN)
                                nxt.append(o)
                            sb = nxt
                            lvl += 1
                        halvings_and_reduce(sb[0], _CHUNK,
                                            sbOut[:, m:m + 1])
                # split the result DMA so the first half ships while the
                # last tiles still compute
                nc.sync.dma_start(dram[f"out{d}"][:, :MT // 2],
                                  sbOut[:, :MT // 2])
                nc.sync.dma_start(dram[f"out{d}"][:, MT // 2:],
                                  sbOut[:, MT // 2:])

    nc.compile()
    return nc


def _get_program(n_pts=_N):
    if n_pts not in _prog_cache:
        _prog_cache[n_pts] = _build_program(n_pts)
    return _prog_cache[n_pts]


def _split_bf16(x32):
    """x32 fp32 -> (hi, lo) bf16 with hi+lo ~= x to ~2^-18 rel."""
    hi = x32.astype(_BF16)
    lo = (x32 - hi.astype(np.float32)).astype(_BF16)
    return hi, lo


def _split3(x64):
    """fp64 vector -> 3 bf16 terms summing to x to ~2^-27 rel."""
    t0 = x64.astype(_BF16)
    r = x64 - t0.astype(np.float64)
    t1 = r.astype(_BF16)
    r2 = r - t1.astype(np.float64)
    t2 = r2.astype(_BF16)
    return t0, t1, t2


def _features(q, c, n_pts):
    """Feature tensors for one direction.

    q: query points  [3, N] fp32; c: candidate points [3, N] fp32.
    Returns (W [18, N] bf16, R [18, N] bf16) with
      (W.T @ R)[i, j] ~= |q~_i - c~_j|^2
    with ~ the bf16-split (hi+lo) values, exact to ~2e-6.
    """
    q_hi, q_lo = _split_bf16(q)
    c_hi, c_lo = _split_bf16(c)
    q_t = q_hi.astype(np.float32) + q_lo.astype(np.float32)
    c_t = c_hi.astype(np.float32) + c_lo.astype(np.float32)

    U = (c_t.astype(np.float64) ** 2).sum(axis=0)   # candidate norms
    u0, u1, u2 = _split3(U)
    V = (q_t.astype(np.float64) ** 2).sum(axis=0)   # query norms
    v0, v1, v2 = _split3(V)

    m2q_hi = (-2.0 * q_hi.astype(np.float32)).astype(_BF16)
    m2q_lo = (-2.0 * q_lo.astype(np.float32)).astype(_BF16)
    ones = np.ones(n_pts, dtype=_BF16)

    Wg = np.concatenate([
        m2q_hi, m2q_hi, m2q_lo, m2q_lo,
        np.stack([ones, ones, ones]),
        np.stack([v0, v1, v2]),
    ], axis=0).astype(_BF16)              # [18, N]
    Rg = np.concatenate([
        c_hi, c_lo, c_hi, c_lo,
        np.stack([u0, u1, u2]),
        np.stack([ones, ones, ones]),
    ], axis=0).astype(_BF16)              # [18, N]

    return Wg, Rg


def kernel(srcs, tgts):
    import concourse.bass_utils as bass_utils

    srcs = np.asarray(srcs, dtype=np.float32)
    tgts = np.asarray(tgts, dtype=np.float32)
    B = srcs.shape[0]
    assert srcs.shape == (B, _D, _N) and tgts.shape == (B, _D, _N)

    nc = _get_program()

    in_maps = []
    for b in range(B):
        s = srcs[b]
        t = tgts[b]
        W1, R1 = _features(s, t, _N)   # dir1: queries = sources
        W2, R2 = _features(t, s, _N)   # dir2: queries = targets
        in_maps.append({"w1": W1, "r1": R1, "w2": W2, "r2": R2})

    res = None
    for attempt in range(3):
        try:
            res = bass_utils.run_bass_kernel_spmd(
                nc, in_maps, core_ids=list(range(_NCORES)),
                trace=TRACE, trace_cores=TRACE_CORES if TRACE else None,
            )
            break
        except Exception:
            # transient NRT/device hiccups have been observed; retry
            if attempt == 2:
                raise
            import time
            time.sleep(3.0)
    global LAST_RESULTS
    LAST_RESULTS = res

    total = 0.0
    for b in range(B):
        out1 = res.results[b]["out1"]   # [128, 32]; query i = m*128 + p
        out2 = res.results[b]["out2"]
        # reference: min(P, axis=1).mean() -> per-target mins (dir2);
        #            min(P, axis=2).mean() -> per-source mins (dir1)
        total += (out2.astype(np.float64).mean()
                  + out1.astype(np.float64).mean())

    return np.float32(total / B)


# revision 34
# speedup vs baseline: 1.1760x; 1.1760x over previous
"""Chamfer-distance (CDLoss) Trainium2 Bass kernel.

Problem: srcs, tgts [B=8, D=3, N=4096] fp32.
  P[b,i,j] = |s_i|^2 + |t_j|^2 - 2 s_i.t_j
  out = min(P, axis=1).mean() + min(P, axis=2).mean()   (scalar fp32)

Strategy (data-parallel over B across 8 NeuronCores, one batch per core):
  Two "directions" per core, each a 4096x4096 implicit distance matrix:
    dir1: for each source i, min over targets j of d^2(s_i, t_j)
    dir2: for each target j, min over sources i of d^2(t_j, s_i)

  Matrix tiles are produced by TensorE matmuls with bf16 hi/lo-split
  features (K=18 rows; query and candidate norms are folded in, so PSUM
  holds d^2 >= 0 exactly to ~1e-6).  The 4 matmuls of each PSUM chunk go
  to 4 different PE row groups (tile_position) so LDWEIGHTS/MATMUL
  overlap and matmuls run concurrently.

  Row-min reduction is split between two engines:
    - "assisted" M-tiles: ScalarE casts PSUM fp32 -> fp16 SBUF (ACTIVATE
      Copy), then VectorE runs a tensor_tensor min tree in fp16 (2x packed
      mode, 2 outputs/cycle) + one small 1x reduce.
    - "pure" M-tiles: VectorE reduces PSUM fp32 directly at 1x.
  The ratio keeps both engines saturated.

  Per-core outputs are 2x[128, 32] row-min matrices; the host averages
  and combines across cores (query norms are already included).
"""

import numpy as np
import ml_dtypes

_BF16 = ml_dtypes.bfloat16

# Problem geometry (hardcoded per contest contract).
_B = 8
_D = 3
_N = 4096
_P = 128              # partitions / queries per M-tile
_K = 18               # feature rows (see _features)
_NCORES = 8
_CHUNK = 2048         # PSUM chunk columns (4 banks)

_prog_cache = {}

# test-harness knobs (the grading harness just calls kernel() and never
# touches these; default is the fast no-trace path)
TRACE = False
TRACE_CORES = [0]
LAST_RESULTS = None

# M-tile flavors: "full" = ACT casts both chunks, DVE runs the fp16 tree;
# "mixed" = ACT casts chunk0 while DVE reduces chunk1 from PSUM directly.
# Ratio tuned so both engines stay saturated (see _is_mixed).
_MIX_NUM, _MIX_DEN = 12, 32


def _is_mixed(m):
    return (m * _MIX_NUM) % _MIX_DEN < _MIX_NUM


def _build_program(n_pts=_N):
    import concourse.mybir as mybir
    import concourse.tile as tile
    from concourse import bacc

    P = _P
    MT = n_pts // P
    K = _K
    NCH = n_pts // _CHUNK          # psum chunks per M-tile
    f32 = mybir.dt.float32
    f16 = mybir.dt.float16
    bf16 = mybir.dt.bfloat16
    MIN = mybir.AluOpType.min

    nc = bacc.Bacc("TRN2", target_bir_lowering=False, debug=False,
                   num_devices=_NCORES)

    dram = {}
    for d in (1, 2):
        dram[f"w{d}"] = nc.dram_tensor(f"w{d}", [K, n_pts], bf16,
                                       kind="ExternalInput")
        dram[f"r{d}"] = nc.dram_tensor(f"r{d}", [K, n_pts], bf16,
                                       kind="ExternalInput")
        dram[f"out{d}"] = nc.dram_tensor(f"out{d}", [P, MT], f32,
                                         kind="ExternalOutput")

    with tile.TileContext(nc) as tc:
        with (
            tc.tile_pool(name="const", bufs=2) as cpool,
            tc.tile_pool(name="work", bufs=6) as wpool,
            tc.tile_pool(name="tree", bufs=6) as tpool,
            tc.tile_pool(name="acc", bufs=2) as apool,
            tc.tile_pool(name="psum", bufs=2, space="PSUM") as ppool,
        ):
            def halvings_and_reduce(cur, w, out_ap):
                """DVE: fp16 halving TTs (2x mode) then a small 1x
                reduce, [P, w] fp16 -> out_ap [P, 1] f32 (column min)."""
                while w > 512:
                    o = tpool.tile([P, w // 2], f16, tag=f"h{w}",
                                   name=f"h{w}")
                    nc.vector.tensor_tensor(
                        o[:], cur[:, :w // 2], cur[:, w // 2:], op=MIN)
                    cur, w = o, w // 2
                nc.vector.tensor_reduce(
                    out_ap, cur[:], axis=mybir.AxisListType.X, op=MIN)

            for d in (1, 2):
                # Inputs are compact [K, N]; replicate into the 4 PE row
                # groups with 4 small DMAs each.
                sbW = cpool.tile([128, n_pts], bf16, tag="sbW")
                sbR = cpool.tile([128, n_pts], bf16, tag="sbR")
                for g in range(4):
                    # direction 1: two HWDGE queues so the startup loads
                    # overlap; direction 2 loads go on the idle sync
                    # queue so they never interrupt ScalarE's casts.
                    r_eng = nc.scalar if d == 1 else nc.sync
                    nc.sync.dma_start(sbW[32 * g:32 * g + K, :],
                                      dram[f"w{d}"][:])
                    r_eng.dma_start(sbR[32 * g:32 * g + K, :],
                                    dram[f"r{d}"][:])
                sbOut = cpool.tile([P, MT], f32, tag="sbOut")

                for m in range(MT):
                    mixed = _is_mixed(m) and NCH == 2
                    starter = (d == 1 and m == 0 and NCH == 2)
                    if starter:
                        # First tile: 1024-col chunks (2 matmuls each) so
                        # the first ScalarE cast fires as early as
                        # possible after the input DMAs land.
                        sb = []
                        for h in range(4):
                            ps = ppool.tile([P, _CHUNK], f32, tag="ps")
                            for q in range(2):
                                g = (2 * h + q) % 4
                                col = 1024 * h + 512 * q
                                nc.tensor.matmul(
                                    ps[:, 512 * q:512 * (q + 1)],
                                    sbW[32 * g:32 * g + K, :P],
                                    sbR[32 * g:32 * g + K, col:col + 512],
                                    start=True, stop=True,
                                    tile_position=(32 * g, 0),
                                )
                            cast = wpool.tile([P, 1024], f16,
                                              tag="scast", name="scast")
                            nc.scalar.copy(cast[:], ps[:, 0:1024])
                            sb.append(cast)
                        lvl = 0
                        while len(sb) > 1:
                            nxt = []
                            for i in range(0, len(sb), 2):
                                o = tpool.tile([P, 1024], f16,
                                               tag=f"st{lvl}_{i}",
                                               name=f"st{lvl}_{i}")
                                nc.vector.tensor_tensor(
                                    o[:], sb[i][:], sb[i + 1][:], op=MIN)
                                nxt.append(o)
                            sb = nxt
                            lvl += 1
                        halvings_and_reduce(sb[0], 1024,
                                            sbOut[:, m:m + 1])
                        continue
                    chunks = []
                    for h in range(NCH):
                        ps = ppool.tile([P, _CHUNK], f32, tag="ps")
                        for q in range(4):
                            # row group rotates per 512-col matmul so
                            # LDWEIGHTS overlaps in-flight MATMULs and
                            # matmuls run concurrently on the PE array.
                            g = q
                            col = _CHUNK * h + 512 * q
                            nc.tensor.matmul(
                                ps[:, 512 * q:512 * (q + 1)],
                                sbW[32 * g:32 * g + K, m * P:(m + 1) * P],
                                sbR[32 * g:32 * g + K, col:col + 512],
                                start=True, stop=True,
                                tile_position=(32 * g, 0),
                            )
                        chunks.append(ps)

                    if mixed:
                        # ACT casts chunk0; DVE reduces chunk1 from PSUM
                        # concurrently; combine the two partial mins.
                        cast = wpool.tile([P, _CHUNK], f16, tag="cast0")
                        nc.scalar.copy(cast[:], chunks[0][:])
                        tmp = apool.tile([P, 2], f32, tag="tmp")
                        nc.vector.tensor_reduce(
                            tmp[:, 1:2], chunks[1][:],
                            axis=mybir.AxisListType.X, op=MIN)
                        halvings_and_reduce(cast, _CHUNK, tmp[:, 0:1])
                        nc.vector.tensor_reduce(
                            sbOut[:, m:m + 1], tmp[:],
                            axis=mybir.AxisListType.X, op=MIN)
                    else:
                        sb = []
                        for h, ps in enumerate(chunks):
                            cast = wpool.tile([P, _CHUNK], f16,
                                              tag=f"cast{h}")
                            nc.scalar.copy(cast[:], ps[:])
                            sb.append(cast)
                        # fold tile pairs at 2x, then halve
                        lvl = 0
                        while len(sb) > 1:
                            nxt = []
                            for i in range(0, len(sb), 2):
                                o = tpool.tile([P, _CHUNK], f16,
                                               tag=f"t{lvl}_{i}")
                                nc.vector.tensor_tensor(
                                    o[:], sb[i][:], sb[i + 1][:], op=MIN)
                                nxt.append(o)
                            sb = nxt
                            lvl += 1
                        halvings_and_reduce(sb[0], _CHUNK,
                                            sbOut[:, m:m + 1])
                # split the result DMA so the first half ships while the
                # last tiles still compute
                nc.sync.dma_start(dram[f"out{d}"][:, :MT // 2],
                                  sbOut[:, :MT // 2])
                nc.sync.dma_start(dram[f"out{d}"][:, MT // 2:],
                                  sbOut[:, MT // 2:])

    nc.compile()
    return nc


def _get_program(n_pts=_N):
    if n_pts not in _prog_cache:
        _prog_cache[n_pts] = _build_program(n_pts)
    return _prog_cache[n_pts]


def _split_bf16(x32):
    """x32 fp32 -> (hi, lo) bf16 with hi+lo ~= x to ~2^-18 rel."""
    hi = x32.astype(_BF16)
    lo = (x32 - hi.astype(np.float32)).astype(_BF16)
    return hi, lo


def _split3(x64):
    """fp64 vector -> 3 bf16 terms summing to x to ~2^-27 rel."""
    t0 = x64.astype(_BF16)
    r = x64 - t0.astype(np.float64)
    t1 = r.astype(_BF16)
    r2 = r - t1.astype(np.float64)
    t2 = r2.astype(_BF16)
    return t0, t1, t2


def _features(q, c, n_pts):
    """Feature tensors for one direction.

    q: query points  [3, N] fp32; c: candidate points [3, N] fp32.
    Returns (W [18, N] bf16, R [18, N] bf16) with
      (W.T @ R)[i, j] ~= |q~_i - c~_j|^2
    with ~ the bf16-split (hi+lo) values, exact to ~2e-6.
    """
    q_hi, q_lo = _split_bf16(q)
    c_hi, c_lo = _split_bf16(c)
    q_t = q_hi.astype(np.float32) + q_lo.astype(np.float32)
    c_t = c_hi.astype(np.float32) + c_lo.astype(np.float32)

    U = (c_t.astype(np.float64) ** 2).sum(axis=0)   # candidate norms
    u0, u1, u2 = _split3(U)
    V = (q_t.astype(np.float64) ** 2).sum(axis=0)   # query norms
    v0, v1, v2 = _split3(V)

    m2q_hi = (-2.0 * q_hi.astype(np.float32)).astype(_BF16)
    m2q_lo = (-2.0 * q_lo.astype(np.float32)).astype(_BF16)
    ones = np.ones(n_pts, dtype=_BF16)

    Wg = np.concatenate([
        m2q_hi, m2q_hi, m2q_lo, m2q_lo,
        np.stack([ones, ones, ones]),
        np.stack([v0, v1, v2]),
    ], axis=0).astype(_BF16)              # [18, N]
    Rg = np.concatenate([
        c_hi, c_lo, c_hi, c_lo,
        np.stack([u0, u1, u2]),
        np.stack([ones, ones, ones]),
    ], axis=0).astype(_BF16)              # [18, N]

    return Wg, Rg


def kernel(srcs, tgts):
    import concourse.bass_utils as bass_utils

    srcs = np.asarray(srcs, dtype=np.float32)
    tgts = np.asarray(tgts, dtype=np.float32)
    B = srcs.shape[0]
    assert srcs.shape == (B, _D, _N) and tgts.shape == (B, _D, _N)

    nc = _get_program()

    in_maps = []
    for b in range(B):
        s = srcs[b]
        t = tgts[b]
        W1, R1 = _features(s, t, _N)   # dir1: queries = sources
        W2, R2 = _features(t, s, _N)   # dir2: queries = targets
        in_maps.append({"w1": W1, "r1": R1, "w2": W2, "r2": R2})

    res = None
    for attempt in range(3):
        try:
            res = bass_utils.run_bass_kernel_spmd(
                nc, in_maps, core_ids=list(range(_NCORES)),
                trace=TRACE, trace_cores=TRACE_CORES if TRACE else None,
            )
            break
        except Exception:
            # transient NRT/device hiccups have been observed; retry
            if attempt == 2:
                raise
            import time
            time.sleep(3.0)
    global LAST_RESULTS
    LAST_RESULTS = res

    total = 0.0
    for b in range(B):
        out1 = res.results[b]["out1"]   # [128, 32]; query i = m*128 + p
        out2 = res.results[b]["out2"]
        # reference: min(P, axis=1).mean() -> per-target mins (dir2);
        #            min(P, axis=2).mean() -> per-source mins (dir1)
        total += (out2.astype(np.float64).mean()
                  + out1.astype(np.float64).mean())

    return np.float32(total / B)


# revision 36
# speedup vs baseline: 1.1777x; 1.0015x over previous
"""Chamfer-distance (CDLoss) Trainium2 Bass kernel.

Problem: srcs, tgts [B=8, D=3, N=4096] fp32.
  P[b,i,j] = |s_i|^2 + |t_j|^2 - 2 s_i.t_j
  out = min(P, axis=1).mean() + min(P, axis=2).mean()   (scalar fp32)

Strategy (data-parallel over B across 8 NeuronCores, one batch per core):
  Two "directions" per core, each a 4096x4096 implicit distance matrix:
    dir1: for each source i, min over targets j of d^2(s_i, t_j)
    dir2: for each target j, min over sources i of d^2(t_j, s_i)

  Matrix tiles are produced by TensorE matmuls with bf16 hi/lo-split
  features (K=18 rows; query and candidate norms are folded in, so PSUM
  holds d^2 >= 0 exactly to ~1e-6).  The 4 matmuls of each PSUM chunk go
  to 4 different PE row groups (tile_position) so LDWEIGHTS/MATMUL
  overlap and matmuls run concurrently.

  Row-min reduction is split between two engines:
    - "assisted" M-tiles: ScalarE casts PSUM fp32 -> fp16 SBUF (ACTIVATE
      Copy), then VectorE runs a tensor_tensor min tree in fp16 (2x packed
      mode, 2 outputs/cycle) + one small 1x reduce.
    - "pure" M-tiles: VectorE reduces PSUM fp32 directly at 1x.
  The ratio keeps both engines saturated.

  Per-core outputs are 2x[128, 32] row-min matrices; the host averages
  and combines across cores (query norms are already included).
"""

import numpy as np
import ml_dtypes

_BF16 = ml_dtypes.bfloat16

# Problem geometry (hardcoded per contest contract).
_B = 8
_D = 3
_N = 4096
_P = 128              # partitions / queries per M-tile
_K = 18               # feature rows (see _features)
_NCORES = 8
_CHUNK = 2048         # PSUM chunk columns (4 banks)

_prog_cache = {}

# test-harness knobs (the grading harness just calls kernel() and never
# touches these; default is the fast no-trace path)
TRACE = False
TRACE_CORES = [0]
LAST_RESULTS = None

# M-tile flavors: "full" = ACT casts both chunks, DVE runs the fp16 tree;
# "mixed" = ACT casts chunk0 while DVE reduces chunk1 from PSUM directly.
# Ratio tuned so both engines stay saturated (see _is_mixed).
_MIX_NUM, _MIX_DEN = 12, 32


def _is_mixed(m):
    return (m * _MIX_NUM) % _MIX_DEN < _MIX_NUM


def _build_program(n_pts=_N):
    import concourse.mybir as mybir
    import concourse.tile as tile
    from concourse import bacc

    P = _P
    MT = n_pts // P
    K = _K
    NCH = n_pts // _CHUNK          # psum chunks per M-tile
    f32 = mybir.dt.float32
    f16 = mybir.dt.float16
    bf16 = mybir.dt.bfloat16
    MIN = mybir.AluOpType.min

    nc = bacc.Bacc("TRN2", target_bir_lowering=False, debug=False,
                   num_devices=_NCORES)

    dram = {}
    for d in (1, 2):
        dram[f"w{d}"] = nc.dram_tensor(f"w{d}", [K, n_pts], bf16,
                                       kind="ExternalInput")
        dram[f"r{d}"] = nc.dram_tensor(f"r{d}", [K, n_pts], bf16,
                                       kind="ExternalInput")
        dram[f"out{d}"] = nc.dram_tensor(f"out{d}", [P, MT], f32,
                                         kind="ExternalOutput")

    with tile.TileContext(nc) as tc:
        with (
            tc.tile_pool(name="const", bufs=2) as cpool,
            tc.tile_pool(name="work", bufs=6) as wpool,
            tc.tile_pool(name="tree", bufs=6) as tpool,
            tc.tile_pool(name="acc", bufs=4) as apool,
            tc.tile_pool(name="psum", bufs=2, space="PSUM") as ppool,
        ):
            def halvings_and_reduce(cur, w, out_ap):
                """DVE: fp16 halving TTs (2x mode) then a small 1x
                reduce, [P, w] fp16 -> out_ap [P, 1] f32 (column min)."""
                while w > 512:
                    o = tpool.tile([P, w // 2], f16, tag=f"h{w}",
                                   name=f"h{w}")
                    nc.vector.tensor_tensor(
                        o[:], cur[:, :w // 2], cur[:, w // 2:], op=MIN)
                    cur, w = o, w // 2
                nc.vector.tensor_reduce(
                    out_ap, cur[:], axis=mybir.AxisListType.X, op=MIN)

            # Prologue: load BOTH directions' inputs up front (cpool
            # bufs=2 holds them side by side).  Direction 1 uses two
            # HWDGE queues so its loads overlap; direction 2's follow on
            # the sync queue, well before they are needed and before the
            # out1 DMA can block that queue.
            sbWs, sbRs = {}, {}
            for d in (1, 2):
                sbW = cpool.tile([128, n_pts], bf16, tag="sbW",
                                 name=f"sbW{d}")
                sbR = cpool.tile([128, n_pts], bf16, tag="sbR",
                                 name=f"sbR{d}")
                r_eng = nc.scalar if d == 1 else nc.sync
                for g in range(4):
                    nc.sync.dma_start(sbW[32 * g:32 * g + K, :],
                                      dram[f"w{d}"][:])
                    r_eng.dma_start(sbR[32 * g:32 * g + K, :],
                                    dram[f"r{d}"][:])
                sbWs[d], sbRs[d] = sbW, sbR

            for d in (1, 2):
                sbW, sbR = sbWs[d], sbRs[d]
                sbOut = cpool.tile([P, MT], f32, tag="sbOut")

                for m in range(MT):
                    mixed = _is_mixed(m) and NCH == 2
                    starter = (d == 1 and m == 0 and NCH == 2)
                    if starter:
                        # First tile: 1024-col chunks (2 matmuls each) so
                        # the first ScalarE cast fires as early as
                        # possible after the input DMAs land.
                        sb = []
                        for h in range(4):
                            ps = ppool.tile([P, _CHUNK], f32, tag="ps")
                            for q in range(2):
                                g = (2 * h + q) % 4
                                col = 1024 * h + 512 * q
                                nc.tensor.matmul(
                                    ps[:, 512 * q:512 * (q + 1)],
                                    sbW[32 * g:32 * g + K, :P],
                                    sbR[32 * g:32 * g + K, col:col + 512],
                                    start=True, stop=True,
                                    tile_position=(32 * g, 0),
                                )
                            cast = wpool.tile([P, 1024], f16,
                                              tag="scast", name="scast")
                            nc.scalar.copy(cast[:], ps[:, 0:1024])
                            sb.append(cast)
                        lvl = 0
                        while len(sb) > 1:
                            nxt = []
                            for i in range(0, len(sb), 2):
                                o = tpool.tile([P, 1024], f16,
                                               tag=f"st{lvl}_{i}",
                                               name=f"st{lvl}_{i}")
                                nc.vector.tensor_tensor(
                                    o[:], sb[i][:], sb[i + 1][:], op=MIN)
                                nxt.append(o)
                            sb = nxt
                            lvl += 1
                        halvings_and_reduce(sb[0], 1024,
                                            sbOut[:, m:m + 1])
                        continue
                    chunks = []
                    for h in range(NCH):
                        ps = ppool.tile([P, _CHUNK], f32, tag="ps")
                        for q in range(4):
                            # row group rotates per 512-col matmul so
                            # LDWEIGHTS overlaps in-flight MATMULs and
                            # matmuls run concurrently on the PE array.
                            g = q
                            col = _CHUNK * h + 512 * q
                            nc.tensor.matmul(
                                ps[:, 512 * q:512 * (q + 1)],
                                sbW[32 * g:32 * g + K, m * P:(m + 1) * P],
                                sbR[32 * g:32 * g + K, col:col + 512],
                                start=True, stop=True,
                                tile_position=(32 * g, 0),
                            )
                        chunks.append(ps)

                    if mixed:
                        # ACT casts chunk0; DVE reduces chunk1 from PSUM
                        # concurrently; combine the two partial mins.
                        cast = wpool.tile([P, _CHUNK], f16, tag="cast0")
                        nc.scalar.copy(cast[:], chunks[0][:])
                        tmp = apool.tile([P, 2], f32, tag="tmp")
                        nc.vector.tensor_reduce(
                            tmp[:, 1:2], chunks[1][:],
                            axis=mybir.AxisListType.X, op=MIN)
                        halvings_and_reduce(cast, _CHUNK, tmp[:, 0:1])
                        nc.vector.tensor_reduce(
                            sbOut[:, m:m + 1], tmp[:],
                            axis=mybir.AxisListType.X, op=MIN)
                    else:
                        sb = []
                        for h, ps in enumerate(chunks):
                            cast = wpool.tile([P, _CHUNK], f16,
                                              tag=f"cast{h}")
                            nc.scalar.copy(cast[:], ps[:])
                            sb.append(cast)
                        # fold tile pairs at 2x, then halve
                        lvl = 0
                        while len(sb) > 1:
                            nxt = []
                            for i in range(0, len(sb), 2):
                                o = tpool.tile([P, _CHUNK], f16,
                                               tag=f"t{lvl}_{i}")
                                nc.vector.tensor_tensor(
                                    o[:], sb[i][:], sb[i + 1][:], op=MIN)
                                nxt.append(o)
                            sb = nxt
                            lvl += 1
                        halvings_and_reduce(sb[0], _CHUNK,
                                            sbOut[:, m:m + 1])
                # split the result DMA so the first half ships while the
                # last tiles still compute
                nc.sync.dma_start(dram[f"out{d}"][:, :MT // 2],
                                  sbOut[:, :MT // 2])
                nc.sync.dma_start(dram[f"out{d}"][:, MT // 2:],
                                  sbOut[:, MT // 2:])

    nc.compile()
    return nc


def _get_program(n_pts=_N):
    if n_pts not in _prog_cache:
        _prog_cache[n_pts] = _build_program(n_pts)
    return _prog_cache[n_pts]


def _split_bf16(x32):
    """x32 fp32 -> (hi, lo) bf16 with hi+lo ~= x to ~2^-18 rel."""
    hi = x32.astype(_BF16)
    lo = (x32 - hi.astype(np.float32)).astype(_BF16)
    return hi, lo


def _split3(x64):
    """fp64 vector -> 3 bf16 terms summing to x to ~2^-27 rel."""
    t0 = x64.astype(_BF16)
    r = x64 - t0.astype(np.float64)
    t1 = r.astype(_BF16)
    r2 = r - t1.astype(np.float64)
    t2 = r2.astype(_BF16)
    return t0, t1, t2


def _features(q, c, n_pts):
    """Feature tensors for one direction.

    q: query points  [3, N] fp32; c: candidate points [3, N] fp32.
    Returns (W [18, N] bf16, R [18, N] bf16) with
      (W.T @ R)[i, j] ~= |q~_i - c~_j|^2
    with ~ the bf16-split (hi+lo) values, exact to ~2e-6.
    """
    q_hi, q_lo = _split_bf16(q)
    c_hi, c_lo = _split_bf16(c)
    q_t = q_hi.astype(np.float32) + q_lo.astype(np.float32)
    c_t = c_hi.astype(np.float32) + c_lo.astype(np.float32)

    U = (c_t.astype(np.float64) ** 2).sum(axis=0)   # candidate norms
    u0, u1, u2 = _split3(U)
    V = (q_t.astype(np.float64) ** 2).sum(axis=0)   # query norms
    v0, v1, v2 = _split3(V)

    m2q_hi = (-2.0 * q_hi.astype(np.float32)).astype(_BF16)
    m2q_lo = (-2.0 * q_lo.astype(np.float32)).astype(_BF16)
    ones = np.ones(n_pts, dtype=_BF16)

    Wg = np.concatenate([
        m2q_hi, m2q_hi, m2q_lo, m2q_lo,
        np.stack([ones, ones, ones]),
        np.stack([v0, v1, v2]),
    ], axis=0).astype(_BF16)              # [18, N]
    Rg = np.concatenate([
        c_hi, c_lo, c_hi, c_lo,
        np.stack([u0, u1, u2]),
        np.stack([ones, ones, ones]),
    ], axis=0).astype(_BF16)              # [18, N]

    return Wg, Rg


def kernel(srcs, tgts):
    import concourse.bass_utils as bass_utils

    srcs = np.asarray(srcs, dtype=np.float32)
    tgts = np.asarray(tgts, dtype=np.float32)
    B = srcs.shape[0]
    assert srcs.shape == (B, _D, _N) and tgts.shape == (B, _D, _N)

    nc = _get_program()

    in_maps = []
    for b in range(B):
        s = srcs[b]
        t = tgts[b]
        W1, R1 = _features(s, t, _N)   # dir1: queries = sources
        W2, R2 = _features(t, s, _N)   # dir2: queries = targets
        in_maps.append({"w1": W1, "r1": R1, "w2": W2, "r2": R2})

    res = None
    for attempt in range(3):
        try:
            res = bass_utils.run_bass_kernel_spmd(
                nc, in_maps, core_ids=list(range(_NCORES)),
                trace=TRACE, trace_cores=TRACE_CORES if TRACE else None,
            )
            break
        except Exception:
            # transient NRT/device hiccups have been observed; retry
            if attempt == 2:
                raise
            import time
            time.sleep(3.0)
    global LAST_RESULTS
    LAST_RESULTS = res

    total = 0.0
    for b in range(B):
        out1 = res.results[b]["out1"]   # [128, 32]; query i = m*128 + p
        out2 = res.results[b]["out2"]
        # reference: min(P, axis=1).mean() -> per-target mins (dir2);
        #            min(P, axis=2).mean() -> per-source mins (dir1)
        total += (out2.astype(np.float64).mean()
                  + out1.astype(np.float64).mean())

    return np.float32(total / B)
